# revision 12
# baseline (speedup 1.0000x reference)
"""Bidirectional selective-scan SSM (CausalMolSSM) on 8 TRN2 NeuronCores.

Strategy:
  Phase 1 (L-sharded): each core owns 256 sequence rows (+conv halo) and
    computes, in transposed feature-major layout, the in-proj, depthwise
    causal conv, silu, x-proj, dt-proj for both directions.
  AllToAll #1 + AllGather: redistribute dt / x_conv to channel-sharded
    layout (128 channels/core, full L=2048), broadcast B/C.
  Phase 2 (channel-sharded): build dA=exp(dt*A) and dBu=dt*x*B on lanes
    (d,sigma), run the linear recurrence with tensor_tensor_scan along L,
    contract over sigma with a selection matmul, add D*x term.
  AllToAll #2: bring y back to L-sharded layout.
  Phase 3 (L-sharded): gate with silu(z), out-proj, concat, fusion proj.
  The backward direction is handled by the same cores via index reversal
  (core c owns reversed-chunk 7-c, which uses the same x rows as chunk c).
"""
import sys
sys.path.insert(0, '/opt/trn_rl_repo')
import numpy as np
import ml_dtypes

D_MODEL, D_STATE, D_CONV, L = 512, 16, 4, 2048
DI = 1024
NCORES = 8
LC = L // NCORES            # 256
HALO = LC + 6               # 262
DSH = DI // NCORES          # 128 channels per core
NT = DSH * D_STATE // 128   # 16 lane tiles per direction
MQ = 512                    # phase-2 free-dim chunk (one PSUM bank)
NQ = L // MQ                # 4

BF16 = ml_dtypes.bfloat16

# softplus(u) = silu(u) + P(min(u^2, 2.25)); even-part Chebyshev fit, |u|<=1.5
SP_C = (0.6931054730054999, -0.12461714435972031,
        0.014817950932472132, -0.0011165576908336256)

# engine assignment for the per-lane-tile y multiply (tunable)
YMUL_GPSIMD_MOD = 2   # t % YMUL_GPSIMD_MOD == 1 -> gpsimd


def build_bass():
    import concourse.bass as bass
    import concourse.bacc as bacc
    import concourse.tile as tile
    import concourse.mybir as mybir

    dt = mybir.dt
    Alu = mybir.AluOpType
    Act = mybir.ActivationFunctionType

    nc = bacc.Bacc("TRN2", target_bir_lowering=False, debug=False,
                   enable_asserts=True, num_devices=NCORES)

    f32, f32r, bf = dt.float32, dt.float32r, dt.bfloat16

    # ---------------- DRAM I/O ----------------
    xT = nc.dram_tensor("xT", [D_MODEL, HALO], bf, kind="ExternalInput")
    din = {}
    for d in ("f", "b"):
        din[f"inW_{d}"] = nc.dram_tensor(f"inW_{d}", [D_MODEL, 2 * DI], bf, kind="ExternalInput")
        din[f"xpW_{d}"] = nc.dram_tensor(f"xpW_{d}", [DI, DI + 2 * D_STATE], bf, kind="ExternalInput")
        din[f"dtW_{d}"] = nc.dram_tensor(f"dtW_{d}", [DI, DI], bf, kind="ExternalInput")
        din[f"outW_{d}"] = nc.dram_tensor(f"outW_{d}", [DI, D_MODEL], bf, kind="ExternalInput")
        din[f"inbx_{d}"] = nc.dram_tensor(f"inbx_{d}", [128, 8], f32, kind="ExternalInput")
        din[f"inbz_{d}"] = nc.dram_tensor(f"inbz_{d}", [128, 8], f32, kind="ExternalInput")
        din[f"xpbd_{d}"] = nc.dram_tensor(f"xpbd_{d}", [128, 8], f32, kind="ExternalInput")
        din[f"xpbbc_{d}"] = nc.dram_tensor(f"xpbbc_{d}", [32, 1], f32, kind="ExternalInput")
        din[f"dtb_{d}"] = nc.dram_tensor(f"dtb_{d}", [128, 8], f32, kind="ExternalInput")
        din[f"outb_{d}"] = nc.dram_tensor(f"outb_{d}", [128, 4], f32, kind="ExternalInput")
        din[f"convw_{d}"] = nc.dram_tensor(f"convw_{d}", [128, 32], f32, kind="ExternalInput")
        din[f"convb_{d}"] = nc.dram_tensor(f"convb_{d}", [128, 8], f32, kind="ExternalInput")
    fusW = nc.dram_tensor("fusW", [2 * D_MODEL, D_MODEL], bf, kind="ExternalInput")
    fusb = nc.dram_tensor("fusb", [128, 4], f32, kind="ExternalInput")
    Alan = nc.dram_tensor("Alan", [128, NT], f32, kind="ExternalInput")
    Dpl = nc.dram_tensor("Dpl", [128, 1], f32, kind="ExternalInput")
    E128m = nc.dram_tensor("E128m", [128, 16 * 128], f32r, kind="ExternalInput")
    E16m = nc.dram_tensor("E16m", [16, 128], f32r, kind="ExternalInput")
    SEL128m = nc.dram_tensor("SEL128m", [128, 16 * 128], bf, kind="ExternalInput")
    outT = nc.dram_tensor("outT", [D_MODEL, LC], f32, kind="ExternalOutput")

    RG = [list(range(NCORES))]

    with tile.TileContext(nc) as tc:
        with tc.tile_pool(name="dram", bufs=1, space="DRAM") as dram, \
             tc.tile_pool(name="persist", bufs=1) as pp, \
             tc.tile_pool(name="const", bufs=1) as cp:

            c1_in = [dram.tile([NCORES, 2, 128, LC], f32r, tag=f"c1in{i}", name=f"c1in{i}")
                     for i in range(2)]
            c1_out = [dram.tile([NCORES, 2, 128, LC], f32r, tag=f"c1out{i}", name=f"c1out{i}")
                      for i in range(2)]
            ag_in = [dram.tile([32, LC], f32r, tag=f"agin{i}", name=f"agin{i}") for i in range(2)]
            ag_out = [dram.tile([NCORES, 32, LC], f32r, tag=f"agout{i}", name=f"agout{i}")
                      for i in range(2)]
            c2_in = [dram.tile([NCORES, 1, 128, LC], bf, tag=f"c2in{i}", name=f"c2in{i}")
                     for i in range(2)]
            c2_out = [dram.tile([NCORES, 1, 128, LC], bf, tag=f"c2out{i}", name=f"c2out{i}")
                      for i in range(2)]

            # constants
            e128 = cp.tile([128, 16 * 128], f32r, tag="e128")
            e16 = cp.tile([16, 128], f32r, tag="e16")
            sel128 = cp.tile([128, 16 * 128], bf, tag="sel128")
            alan = cp.tile([128, NT], f32, tag="alan")
            dpl = cp.tile([128, 1], f32, tag="dpl")
            nc.sync.dma_start(e128[:], E128m[:])
            nc.sync.dma_start(e16[:], E16m[:])
            nc.sync.dma_start(sel128[:], SEL128m[:])
            nc.sync.dma_start(alan[:], Alan[:])
            nc.sync.dma_start(dpl[:], Dpl[:])

            zs = {}   # persistent silu(z) tiles, (128, LC) bf16, [dir][m]

            # prefetch phase-3 weights early (DMA overlaps phases 1-2)
            p3w = {}
            for didx, d in enumerate(("f", "b")):
                ob = pp.tile([128, 4], f32, tag=f"outb{d}")
                nc.sync.dma_start(ob[:], din[f"outb_{d}"][:])
                p3w[("outb", d)] = ob
                for k in range(8):
                    t = pp.tile([128, D_MODEL], bf, tag=f"outw{d}{k}")
                    nc.sync.dma_start(t[:], din[f"outW_{d}"][128 * k:128 * (k + 1), :])
                    p3w[("outw", d, k)] = t
            fbt = pp.tile([128, 4], f32, tag="fusb")
            nc.sync.dma_start(fbt[:], fusb[:])
            fwt = []
            for k in range(8):
                t = pp.tile([128, D_MODEL], bf, tag=f"fw{k}")
                nc.sync.dma_start(t[:], fusW[128 * k:128 * (k + 1), :])
                fwt.append(t)

            # ================= PHASE 1 =================
            for didx, d in enumerate(("f", "b")):
                off = 0 if d == "f" else 3
                with tc.tile_pool(name=f"p1w_{d}", bufs=1) as wp, \
                     tc.tile_pool(name=f"p1a_{d}", bufs=1) as ap_, \
                     tc.tile_pool(name=f"p1ps_{d}", bufs=4, space="PSUM") as ps1, \
                     tc.tile_pool(name=f"p1sc_{d}", bufs=3) as scp:

                    # biases
                    inbx = scp.tile([128, 8], f32, tag="inbx")
                    inbz = scp.tile([128, 8], f32, tag="inbz")
                    xpbd = scp.tile([128, 8], f32, tag="xpbd")
                    xpbbc = scp.tile([32, 1], f32, tag="xpbbc")
                    dtb = scp.tile([128, 8], f32, tag="dtb")
                    convw = scp.tile([128, 32], f32, tag="convw")
                    convb = scp.tile([128, 8], f32, tag="convb")
                    nc.sync.dma_start(inbx[:], din[f"inbx_{d}"][:])
                    nc.sync.dma_start(inbz[:], din[f"inbz_{d}"][:])
                    nc.sync.dma_start(xpbd[:], din[f"xpbd_{d}"][:])
                    nc.sync.dma_start(xpbbc[:], din[f"xpbbc_{d}"][:])
                    nc.sync.dma_start(dtb[:], din[f"dtb_{d}"][:])
                    nc.sync.dma_start(convw[:], din[f"convw_{d}"][:])
                    nc.sync.dma_start(convb[:], din[f"convb_{d}"][:])

                    # x tiles
                    xsb = []
                    for k in range(4):
                        t = ap_.tile([128, HALO], bf, tag=f"x{k}")
                        nc.sync.dma_start(t[:], xT[128 * k:128 * (k + 1), :])
                        xsb.append(t)

                    # in-proj weights
                    inw = []
                    for k in range(4):
                        t = wp.tile([128, 2 * DI], bf, tag=f"inw{k}")
                        nc.sync.dma_start(t[:], din[f"inW_{d}"][128 * k:128 * (k + 1), :])
                        inw.append(t)

                    xs = []     # pre-conv x_ssm tiles (128, HALO) f32
                    for m in range(16):
                        px = ps1.tile([128, HALO], f32, tag="p1")
                        for k in range(4):
                            nc.tensor.matmul(px[:], inw[k][:, 128 * m:128 * (m + 1)],
                                             xsb[k][:], start=(k == 0), stop=(k == 3))
                        if m < 8:
                            t = ap_.tile([128, HALO], f32, tag=f"xs{m}")
                            nc.scalar.activation(t[:], px[:], Act.Identity,
                                                 bias=inbx[:, m:m + 1])
                            xs.append(t)
                        else:
                            zt = pp.tile([128, LC], bf, tag=f"z{d}{m - 8}")
                            nc.scalar.activation(zt[:], px[:, 3:3 + LC], Act.Silu,
                                                 bias=inbz[:, m - 8:m - 7])
                            zs[(d, m - 8)] = zt

                    # depthwise causal conv + silu
                    xconv = []
                    silu_x = []
                    for m in range(8):
                        a0 = ap_.tile([128, LC], f32, tag="cacc0")
                        nc.vector.tensor_scalar(a0[:], xs[m][:, off:off + LC],
                                                convw[:, 4 * m:4 * m + 1],
                                                convb[:, m:m + 1],
                                                Alu.mult, Alu.add)
                        a1 = ap_.tile([128, LC], f32, tag="cacc1")
                        nc.vector.scalar_tensor_tensor(a1[:], xs[m][:, off + 1:off + 1 + LC],
                                                       convw[:, 4 * m + 1:4 * m + 2], a0[:],
                                                       Alu.mult, Alu.add)
                        a2 = ap_.tile([128, LC], f32, tag="cacc2")
                        nc.vector.scalar_tensor_tensor(a2[:], xs[m][:, off + 2:off + 2 + LC],
                                                       convw[:, 4 * m + 2:4 * m + 3], a1[:],
                                                       Alu.mult, Alu.add)
                        xc = ap_.tile([128, LC], f32r, tag=f"xc{m}")
                        nc.vector.scalar_tensor_tensor(xc[:], xs[m][:, off + 3:off + 3 + LC],
                                                       convw[:, 4 * m + 3:4 * m + 4], a2[:],
                                                       Alu.mult, Alu.add)
                        xconv.append(xc)
                        sx = ap_.tile([128, LC], bf, tag=f"sx{m}")
                        nc.scalar.activation(sx[:], xc[:], Act.Silu)
                        silu_x.append(sx)
                        nc.sync.dma_start(c1_in[didx][m, 1, :, :], xc[:])

                    # x-proj
                    xpw = []
                    for k in range(8):
                        t = wp.tile([128, DI + 2 * D_STATE], bf, tag=f"xpw{k}")
                        nc.sync.dma_start(t[:], din[f"xpW_{d}"][128 * k:128 * (k + 1), :])
                        xpw.append(t)
                    delta = []
                    for m in range(9):
                        rows = 128 if m < 8 else 32
                        px = ps1.tile([128, LC], f32, tag="p1")
                        for k in range(8):
                            nc.tensor.matmul(px[:rows, :],
                                             xpw[k][:, 128 * m:128 * m + rows],
                                             silu_x[k][:], start=(k == 0), stop=(k == 7))
                        if m < 8:
                            t = ap_.tile([128, LC], bf, tag=f"dl{m}")
                            nc.scalar.activation(t[:], px[:], Act.Identity,
                                                 bias=xpbd[:, m:m + 1])
                            delta.append(t)
                        else:
                            bct = ap_.tile([32, LC], f32r, tag="bc")
                            nc.scalar.activation(bct[:], px[:32, :], Act.Identity,
                                                 bias=xpbbc[:])
                            nc.sync.dma_start(ag_in[didx][:, :], bct[:])

                    # dt-proj + softplus
                    dtw = []
                    for k in range(8):
                        t = wp.tile([128, DI], bf, tag=f"dtw{k}")
                        nc.sync.dma_start(t[:], din[f"dtW_{d}"][128 * k:128 * (k + 1), :])
                        dtw.append(t)
                    for m in range(8):
                        px = ps1.tile([128, LC], f32, tag="p1")
                        for k in range(8):
                            nc.tensor.matmul(px[:], dtw[k][:, 128 * m:128 * (m + 1)],
                                             delta[k][:], start=(k == 0), stop=(k == 7))
                        t = ap_.tile([128, LC], f32r, tag=f"dts{m}")
                        nc.scalar.activation(t[:], px[:], Act.Identity,
                                             bias=dtb[:, m:m + 1])
                        nc.sync.dma_start(c1_in[didx][m, 0, :, :], t[:])

            # ================= COLLECTIVES 1 (per direction) =================
            for i in range(2):
                nc.gpsimd.collective_compute(
                    "AllToAll", Alu.bypass, replica_groups=RG,
                    ins=[c1_in[i][:].opt()], outs=[c1_out[i][:].opt()])
                nc.gpsimd.collective_compute(
                    "AllGather", Alu.bypass, replica_groups=RG,
                    ins=[ag_in[i][:].opt()], outs=[ag_out[i][:].opt()])

            # ================= PHASE 2 =================
            for didx, d in enumerate(("f", "b")):
                rev = (d == "b")
                with tc.tile_pool(name=f"p2_{d}", bufs=1) as p2, \
                     tc.tile_pool(name=f"psA_{d}", bufs=2, space="PSUM") as psA, \
                     tc.tile_pool(name=f"psB_{d}", bufs=2, space="PSUM") as psB, \
                     tc.tile_pool(name=f"psY_{d}", bufs=1, space="PSUM") as psY:

                    xc_m = p2.tile([128, L], f32r, tag="xcm")
                    dt_m = p2.tile([128, L], f32r, tag="dtm")
                    dtx = p2.tile([128, L], f32r, tag="dtx")
                    brep = p2.tile([128, L], f32, tag="brep")
                    crep = p2.tile([128, L], bf, tag="crep")

                    with tc.tile_pool(name=f"p2s_{d}", bufs=1) as sc:
                        u_m = sc.tile([128, L], f32r, tag="um")
                        if rev:
                            u_d = sc.tile([128, L], f32r, tag="ud")
                            xc_d = sc.tile([128, L], f32r, tag="xd")
                            nc.sync.dma_start(
                                u_d[:].rearrange("p (s c) -> p s c", s=NCORES),
                                c1_out[didx][:, 0, :, :].rearrange("s p c -> p s c"))
                            nc.sync.dma_start(
                                xc_d[:].rearrange("p (s c) -> p s c", s=NCORES),
                                c1_out[didx][:, 1, :, :].rearrange("s p c -> p s c"))
                            nc.vector.tensor_copy(u_m[:], u_d[:, ::-1])
                            nc.vector.tensor_copy(xc_m[:], xc_d[:, ::-1])
                        else:
                            nc.sync.dma_start(
                                u_m[:].rearrange("p (s c) -> p s c", s=NCORES),
                                c1_out[didx][:, 0, :, :].rearrange("s p c -> p s c"))
                            nc.sync.dma_start(
                                xc_m[:].rearrange("p (s c) -> p s c", s=NCORES),
                                c1_out[didx][:, 1, :, :].rearrange("s p c -> p s c"))

                        # softplus(u) = silu(u) + P3(min(u^2, 2.25)), chunked
                        for q in range(NQ):
                            sl = slice(MQ * q, MQ * (q + 1))
                            su = sc.tile([128, MQ], f32, tag="sp_su")
                            nc.scalar.activation(su[:], u_m[:, sl], Act.Silu)
                            qq = sc.tile([128, MQ], f32, tag="sp_q")
                            nc.scalar.activation(qq[:], u_m[:, sl], Act.Square)
                            qc = sc.tile([128, MQ], f32, tag="sp_qc")
                            nc.vector.tensor_scalar_min(qc[:], qq[:], 2.25)
                            aa = sc.tile([128, MQ], f32, tag="sp_a")
                            nc.vector.tensor_scalar(aa[:], qc[:], SP_C[1], SP_C[0],
                                                    Alu.mult, Alu.add)
                            bb = sc.tile([128, MQ], f32, tag="sp_b")
                            nc.vector.tensor_scalar(bb[:], qc[:], SP_C[3], SP_C[2],
                                                    Alu.mult, Alu.add)
                            t2 = sc.tile([128, MQ], f32, tag="sp_t2")
                            nc.gpsimd.tensor_tensor(t2[:], qc[:], qc[:], Alu.mult)
                            m1 = sc.tile([128, MQ], f32, tag="sp_m1")
                            nc.gpsimd.tensor_tensor(m1[:], t2[:], bb[:], Alu.mult)
                            s1 = sc.tile([128, MQ], f32, tag="sp_s1")
                            nc.gpsimd.tensor_tensor(s1[:], aa[:], m1[:], Alu.add)
                            nc.vector.tensor_tensor(dt_m[:, sl], s1[:], su[:], Alu.add)
                        nc.vector.tensor_tensor(dtx[:], dt_m[:], xc_m[:], Alu.mult)

                        # broadcast B, C across the 16-sigma partition groups
                        B_m = sc.tile([16, L], f32r, tag="bm")
                        C_m = sc.tile([16, L], f32r, tag="cm")
                        if rev:
                            B_sb = sc.tile([16, L], f32r, tag="bsb")
                            C_sb = sc.tile([16, L], f32r, tag="csb")
                            nc.sync.dma_start(
                                B_sb[:].rearrange("p (s c) -> p s c", s=NCORES),
                                ag_out[didx][:, 0:16, :].rearrange("s p c -> p s c"))
                            nc.sync.dma_start(
                                C_sb[:].rearrange("p (s c) -> p s c", s=NCORES),
                                ag_out[didx][:, 16:32, :].rearrange("s p c -> p s c"))
                            nc.vector.tensor_copy(B_m[:], B_sb[:, ::-1])
                            nc.vector.tensor_copy(C_m[:], C_sb[:, ::-1])
                        else:
                            nc.sync.dma_start(
                                B_m[:].rearrange("p (s c) -> p s c", s=NCORES),
                                ag_out[didx][:, 0:16, :].rearrange("s p c -> p s c"))
                            nc.sync.dma_start(
                                C_m[:].rearrange("p (s c) -> p s c", s=NCORES),
                                ag_out[didx][:, 16:32, :].rearrange("s p c -> p s c"))
                        for q in range(NQ):
                            sl = slice(MQ * q, MQ * (q + 1))
                            pq = psA.tile([128, MQ], f32, tag="pa")
                            nc.tensor.matmul(pq[:], e16[:], B_m[:, sl],
                                             start=True, stop=True)
                            nc.scalar.activation(brep[:, sl], pq[:], Act.Copy)
                            pq2 = psA.tile([128, MQ], f32, tag="pa")
                            nc.tensor.matmul(pq2[:], e16[:], C_m[:, sl],
                                             start=True, stop=True)
                            nc.scalar.activation(crep[:, sl], pq2[:], Act.Copy)

                    ypsum = psY.tile([128, L], f32, tag="ypsum")
                    with tc.tile_pool(name=f"p2t_{d}", bufs=2) as tp:
                        for t in range(NT):
                            dA = tp.tile([128, L], f32, tag="dA")
                            dBu = tp.tile([128, L], f32, tag="dBu")
                            for q in range(NQ):
                                sl = slice(MQ * q, MQ * (q + 1))
                                pa = psA.tile([128, MQ], f32, tag="pa")
                                nc.tensor.matmul(pa[:], e128[:, 128 * t:128 * (t + 1)],
                                                 dt_m[:, sl], start=True, stop=True)
                                nc.scalar.activation(dA[:, sl], pa[:], Act.Exp,
                                                     scale=alan[:, t:t + 1])
                                pb = psB.tile([128, MQ], f32, tag="pb")
                                nc.tensor.matmul(pb[:], e128[:, 128 * t:128 * (t + 1)],
                                                 dtx[:, sl], start=True, stop=True)
                                nc.vector.tensor_tensor(dBu[:, sl], pb[:], brep[:, sl],
                                                        Alu.mult)
                            h = tp.tile([128, L], bf, tag="h")
                            nc.vector.tensor_tensor_scan(h[:], dA[:], dBu[:], 0.0,
                                                         Alu.mult, Alu.add)
                            yp = tp.tile([128, L], bf, tag="yp")
                            nc.gpsimd.tensor_tensor(yp[:], h[:], crep[:], Alu.mult)
                            for q in range(NQ):
                                sl = slice(MQ * q, MQ * (q + 1))
                                nc.tensor.matmul(ypsum[:, sl],
                                                 sel128[:, 128 * t:128 * (t + 1)],
                                                 yp[:, sl],
                                                 start=(t == 0), stop=(t == NT - 1),
                                                 skip_group_check=True)

                    y_sb = p2.tile([128, L], bf, tag="ysb")
                    nc.vector.scalar_tensor_tensor(y_sb[:], xc_m[:], dpl[:], ypsum[:],
                                                   Alu.mult, Alu.add)
                    if rev:
                        y_r = p2.tile([128, L], bf, tag="yr")
                        nc.vector.tensor_copy(y_r[:], y_sb[:, ::-1])
                    else:
                        y_r = y_sb
                    for dst in range(NCORES):
                        nc.sync.dma_start(c2_in[didx][dst, 0, :, :],
                                          y_r[:, LC * dst:LC * (dst + 1)])
                nc.gpsimd.collective_compute(
                    "AllToAll", Alu.bypass, replica_groups=RG,
                    ins=[c2_in[didx][:].opt()], outs=[c2_out[didx][:].opt()])

            # ================= PHASE 3 =================
            cat = []
            with tc.tile_pool(name="p3", bufs=2) as p3, \
                 tc.tile_pool(name="p3c", bufs=1) as p3c, \
                 tc.tile_pool(name="p3ps", bufs=4, space="PSUM") as ps3:
                for didx, d in enumerate(("f", "b")):
                    outb = p3w[("outb", d)]
                    gates = []
                    for m in range(8):
                        y3 = p3.tile([128, LC], bf, tag=f"y3{m}")
                        nc.sync.dma_start(y3[:], c2_out[didx][m, 0, :, :])
                        g = p3.tile([128, LC], bf, tag=f"g{m}")
                        nc.vector.tensor_tensor(g[:], y3[:], zs[(d, m)][:], Alu.mult)
                        gates.append(g)
                    outw = [p3w[("outw", d, k)] for k in range(8)]
                    for m in range(4):
                        po = ps3.tile([128, LC], f32, tag="p3a")
                        for k in range(8):
                            nc.tensor.matmul(po[:], outw[k][:, 128 * m:128 * (m + 1)],
                                             gates[k][:], start=(k == 0), stop=(k == 7))
                        ct = p3c.tile([128, LC], bf, tag=f"cat{didx}{m}")
                        nc.scalar.activation(ct[:], po[:], Act.Identity,
                                             bias=outb[:, m:m + 1])
                        cat.append(ct)
                # fusion
                fb = fbt
                fw = fwt
                for m in range(4):
                    pf = ps3.tile([128, LC], f32, tag="p3b")
                    for k in range(8):
                        nc.tensor.matmul(pf[:], fw[k][:, 128 * m:128 * (m + 1)],
                                         cat[k][:], start=(k == 0), stop=(k == 7))
                    ot = p3.tile([128, LC], f32, tag="ot")
                    nc.scalar.activation(ot[:], pf[:], Act.Identity, bias=fb[:, m:m + 1])
                    nc.sync.dma_start(outT[128 * m:128 * (m + 1), :], ot[:])

    nc.compile()
    return nc


def make_in_maps(inputs):
    x = np.asarray(inputs["x"], np.float32)
    A = -np.exp(np.asarray(inputs["A_log"], np.float32))          # (DI, S)
    Dp = np.asarray(inputs["D_param"], np.float32)

    def bias_tiles(b, ntiles):
        return np.ascontiguousarray(
            np.asarray(b, np.float32).reshape(ntiles, 128).T)

    common = {}
    for d, pre in (("f", "fwd_"), ("b", "bwd_")):
        inW = np.asarray(inputs[pre + "in_W"], np.float32)
        inb = np.asarray(inputs[pre + "in_b"], np.float32)
        cw = np.asarray(inputs[pre + "conv_w"], np.float32)
        if d == "b":
            cw = cw[:, ::-1]
        cb = np.asarray(inputs[pre + "conv_b"], np.float32)
        xpW = np.asarray(inputs[pre + "xp_W"], np.float32)
        xpb = np.asarray(inputs[pre + "xp_b"], np.float32)
        dtW = np.asarray(inputs[pre + "dt_W"], np.float32)
        dtb = np.asarray(inputs[pre + "dt_b"], np.float32)
        outW = np.asarray(inputs[pre + "out_W"], np.float32)
        outb = np.asarray(inputs[pre + "out_b"], np.float32)
        common[f"inW_{d}"] = inW.astype(BF16)
        common[f"inbx_{d}"] = bias_tiles(inb[:DI], 8)
        common[f"inbz_{d}"] = bias_tiles(inb[DI:], 8)
        common[f"convw_{d}"] = np.ascontiguousarray(
            cw.reshape(8, 128, 4).transpose(1, 0, 2).reshape(128, 32))
        common[f"convb_{d}"] = bias_tiles(cb, 8)
        common[f"xpW_{d}"] = xpW.astype(BF16)
        common[f"xpbd_{d}"] = bias_tiles(xpb[:DI], 8)
        common[f"xpbbc_{d}"] = np.ascontiguousarray(xpb[DI:].reshape(32, 1))
        common[f"dtW_{d}"] = dtW.astype(BF16)
        common[f"dtb_{d}"] = bias_tiles(dtb, 8)
        common[f"outW_{d}"] = outW.astype(BF16)
        common[f"outb_{d}"] = bias_tiles(outb, 4)
    common["fusW"] = np.asarray(inputs["fusion_W"], np.float32).astype(BF16)
    common["fusb"] = bias_tiles(np.asarray(inputs["fusion_b"], np.float32), 4)

    p = np.arange(128)
    e128 = np.zeros((128, 16 * 128), np.float32)
    sel128 = np.zeros((128, 16 * 128), np.float32)
    for t in range(16):
        e128[8 * t + p // 16, 128 * t + p] = 1.0
        sel128[p, 128 * t + 8 * t + p // 16] = 1.0
    e16 = np.zeros((16, 128), np.float32)
    e16[p % 16, p] = 1.0
    common["E128m"] = e128
    common["E16m"] = e16
    common["SEL128m"] = sel128.astype(BF16)

    in_maps = []
    for c in range(NCORES):
        m = dict(common)
        r0 = LC * c
        xpad = np.zeros((HALO, D_MODEL), np.float32)
        lo, hi = max(0, r0 - 3), min(L, r0 + LC + 3)
        xpad[lo - (r0 - 3): hi - (r0 - 3)] = x[lo:hi]
        m["xT"] = np.ascontiguousarray(xpad.T).astype(BF16)
        A_sh = A[128 * c:128 * (c + 1)]                      # (128, 16)
        m["Alan"] = np.ascontiguousarray(
            A_sh.reshape(16, 8, 16).transpose(1, 2, 0).reshape(128, NT))
        m["Dpl"] = np.ascontiguousarray(Dp[128 * c:128 * (c + 1)].reshape(128, 1))
        in_maps.append(m)
    return in_maps


_CACHE = {}


def kernel(**inputs):
    from concourse.bass_utils import run_bass_kernel_spmd
    if "nc" not in _CACHE:
        _CACHE["nc"] = build_bass()
    nc = _CACHE["nc"]
    in_maps = make_in_maps(inputs)
    res = run_bass_kernel_spmd(nc, in_maps, list(range(NCORES)))
    outs = [res.results[c]["outT"] for c in range(NCORES)]
    full = np.concatenate(outs, axis=1)      # (512, 2048)
    return np.ascontiguousarray(full.T).astype(np.float32)


# revision 13
# speedup vs baseline: 1.0663x; 1.0663x over previous
"""Bidirectional selective-scan SSM (CausalMolSSM) on 8 TRN2 NeuronCores.

Strategy:
  Phase 1 (L-sharded): each core owns 256 sequence rows (+conv halo) and
    computes, in transposed feature-major layout, the in-proj, depthwise
    causal conv, silu, x-proj, dt-proj for both directions.
  AllToAll #1 + AllGather: redistribute dt / x_conv to channel-sharded
    layout (128 channels/core, full L=2048), broadcast B/C.
  Phase 2 (channel-sharded): build dA=exp(dt*A) and dBu=dt*x*B on lanes
    (d,sigma), run the linear recurrence with tensor_tensor_scan along L,
    contract over sigma with a selection matmul, add D*x term.
  AllToAll #2: bring y back to L-sharded layout.
  Phase 3 (L-sharded): gate with silu(z), out-proj, concat, fusion proj.
  The backward direction is handled by the same cores via index reversal
  (core c owns reversed-chunk 7-c, which uses the same x rows as chunk c).
"""
import sys
sys.path.insert(0, '/opt/trn_rl_repo')
import numpy as np
import ml_dtypes

D_MODEL, D_STATE, D_CONV, L = 512, 16, 4, 2048
DI = 1024
NCORES = 8
LC = L // NCORES            # 256
HALO = LC + 6               # 262
DSH = DI // NCORES          # 128 channels per core
NT = DSH * D_STATE // 128   # 16 lane tiles per direction
MQ = 512                    # phase-2 free-dim chunk (one PSUM bank)
NQ = L // MQ                # 4

BF16 = ml_dtypes.bfloat16

# softplus(u) = silu(u) + P(min(u^2, 2.25)); even-part Chebyshev fit, |u|<=1.5
SP_C = (0.6931054730054999, -0.12461714435972031,
        0.014817950932472132, -0.0011165576908336256)

# engine assignment for the per-lane-tile y multiply (tunable)
YMUL_GPSIMD_MOD = 2   # t % YMUL_GPSIMD_MOD == 1 -> gpsimd


def build_bass():
    import concourse.bass as bass
    import concourse.bacc as bacc
    import concourse.tile as tile
    import concourse.mybir as mybir

    dt = mybir.dt
    Alu = mybir.AluOpType
    Act = mybir.ActivationFunctionType

    nc = bacc.Bacc("TRN2", target_bir_lowering=False, debug=False,
                   enable_asserts=True, num_devices=NCORES)

    f32, f32r, bf = dt.float32, dt.float32r, dt.bfloat16

    # ---------------- DRAM I/O ----------------
    xT = nc.dram_tensor("xT", [D_MODEL, HALO], bf, kind="ExternalInput")
    din = {}
    for d in ("f", "b"):
        din[f"inW_{d}"] = nc.dram_tensor(f"inW_{d}", [D_MODEL, 2 * DI], bf, kind="ExternalInput")
        din[f"xpW_{d}"] = nc.dram_tensor(f"xpW_{d}", [DI, DI + 2 * D_STATE], bf, kind="ExternalInput")
        din[f"dtW_{d}"] = nc.dram_tensor(f"dtW_{d}", [DI, DI], bf, kind="ExternalInput")
        din[f"outW_{d}"] = nc.dram_tensor(f"outW_{d}", [DI, D_MODEL], bf, kind="ExternalInput")
        din[f"inbx_{d}"] = nc.dram_tensor(f"inbx_{d}", [128, 8], f32, kind="ExternalInput")
        din[f"inbz_{d}"] = nc.dram_tensor(f"inbz_{d}", [128, 8], f32, kind="ExternalInput")
        din[f"xpbd_{d}"] = nc.dram_tensor(f"xpbd_{d}", [128, 8], f32, kind="ExternalInput")
        din[f"xpbbc_{d}"] = nc.dram_tensor(f"xpbbc_{d}", [32, 1], f32, kind="ExternalInput")
        din[f"dtb_{d}"] = nc.dram_tensor(f"dtb_{d}", [128, 8], f32, kind="ExternalInput")
        din[f"outb_{d}"] = nc.dram_tensor(f"outb_{d}", [128, 4], f32, kind="ExternalInput")
        din[f"convw_{d}"] = nc.dram_tensor(f"convw_{d}", [128, 32], f32, kind="ExternalInput")
        din[f"convb_{d}"] = nc.dram_tensor(f"convb_{d}", [128, 8], f32, kind="ExternalInput")
    fusW = nc.dram_tensor("fusW", [2 * D_MODEL, D_MODEL], bf, kind="ExternalInput")
    fusb = nc.dram_tensor("fusb", [128, 4], f32, kind="ExternalInput")
    Alan = nc.dram_tensor("Alan", [128, NT], f32, kind="ExternalInput")
    Dpl = nc.dram_tensor("Dpl", [128, 1], f32, kind="ExternalInput")
    E128m = nc.dram_tensor("E128m", [128, 16 * 128], f32r, kind="ExternalInput")
    E16m = nc.dram_tensor("E16m", [16, 128], f32r, kind="ExternalInput")
    SEL128m = nc.dram_tensor("SEL128m", [128, 16 * 128], bf, kind="ExternalInput")
    outT = nc.dram_tensor("outT", [D_MODEL, LC], f32, kind="ExternalOutput")

    RG = [list(range(NCORES))]

    with tile.TileContext(nc) as tc:
        with tc.tile_pool(name="dram", bufs=1, space="DRAM") as dram, \
             tc.tile_pool(name="persist", bufs=1) as pp, \
             tc.tile_pool(name="const", bufs=1) as cp:

            c1_in = [dram.tile([NCORES, 2, 128, LC], f32r, tag=f"c1in{i}", name=f"c1in{i}")
                     for i in range(2)]
            c1_out = [dram.tile([NCORES, 2, 128, LC], f32r, tag=f"c1out{i}", name=f"c1out{i}")
                      for i in range(2)]
            ag_in = [dram.tile([32, LC], f32r, tag=f"agin{i}", name=f"agin{i}") for i in range(2)]
            ag_out = [dram.tile([NCORES, 32, LC], f32r, tag=f"agout{i}", name=f"agout{i}")
                      for i in range(2)]
            c2_in = [dram.tile([NCORES, 1, 128, LC], bf, tag=f"c2in{i}", name=f"c2in{i}")
                     for i in range(2)]
            c2_out = [dram.tile([NCORES, 1, 128, LC], bf, tag=f"c2out{i}", name=f"c2out{i}")
                      for i in range(2)]

            # constants
            e128 = cp.tile([128, 16 * 128], f32r, tag="e128")
            e16 = cp.tile([16, 128], f32r, tag="e16")
            sel128 = cp.tile([128, 16 * 128], bf, tag="sel128")
            alan = cp.tile([128, NT], f32, tag="alan")
            dpl = cp.tile([128, 1], f32, tag="dpl")
            nc.sync.dma_start(e128[:], E128m[:])
            nc.sync.dma_start(e16[:], E16m[:])
            nc.sync.dma_start(sel128[:], SEL128m[:])
            nc.sync.dma_start(alan[:], Alan[:])
            nc.sync.dma_start(dpl[:], Dpl[:])

            zs = {}   # persistent silu(z) tiles, (128, LC) bf16, [dir][m]

            # prefetch phase-3 weights early (DMA overlaps phases 1-2)
            p3w = {}
            for didx, d in enumerate(("f", "b")):
                ob = pp.tile([128, 4], f32, tag=f"outb{d}")
                nc.sync.dma_start(ob[:], din[f"outb_{d}"][:])
                p3w[("outb", d)] = ob
                for k in range(8):
                    t = pp.tile([128, D_MODEL], bf, tag=f"outw{d}{k}")
                    nc.sync.dma_start(t[:], din[f"outW_{d}"][128 * k:128 * (k + 1), :])
                    p3w[("outw", d, k)] = t
            fbt = pp.tile([128, 4], f32, tag="fusb")
            nc.sync.dma_start(fbt[:], fusb[:])
            fwt = []
            for k in range(8):
                t = pp.tile([128, D_MODEL], bf, tag=f"fw{k}")
                nc.sync.dma_start(t[:], fusW[128 * k:128 * (k + 1), :])
                fwt.append(t)

            # ================= PHASE 1 =================
            for didx, d in enumerate(("f", "b")):
                off = 0 if d == "f" else 3
                with tc.tile_pool(name=f"p1w_{d}", bufs=1) as wp, \
                     tc.tile_pool(name=f"p1a_{d}", bufs=1) as ap_, \
                     tc.tile_pool(name=f"p1ps_{d}", bufs=4, space="PSUM") as ps1, \
                     tc.tile_pool(name=f"p1sc_{d}", bufs=3) as scp:

                    # biases
                    inbx = scp.tile([128, 8], f32, tag="inbx")
                    inbz = scp.tile([128, 8], f32, tag="inbz")
                    xpbd = scp.tile([128, 8], f32, tag="xpbd")
                    xpbbc = scp.tile([32, 1], f32, tag="xpbbc")
                    dtb = scp.tile([128, 8], f32, tag="dtb")
                    convw = scp.tile([128, 32], f32, tag="convw")
                    convb = scp.tile([128, 8], f32, tag="convb")
                    nc.sync.dma_start(inbx[:], din[f"inbx_{d}"][:])
                    nc.sync.dma_start(inbz[:], din[f"inbz_{d}"][:])
                    nc.sync.dma_start(xpbd[:], din[f"xpbd_{d}"][:])
                    nc.sync.dma_start(xpbbc[:], din[f"xpbbc_{d}"][:])
                    nc.sync.dma_start(dtb[:], din[f"dtb_{d}"][:])
                    nc.sync.dma_start(convw[:], din[f"convw_{d}"][:])
                    nc.sync.dma_start(convb[:], din[f"convb_{d}"][:])

                    # x tiles
                    xsb = []
                    for k in range(4):
                        t = ap_.tile([128, HALO], bf, tag=f"x{k}")
                        nc.sync.dma_start(t[:], xT[128 * k:128 * (k + 1), :])
                        xsb.append(t)

                    # in-proj weights
                    inw = []
                    for k in range(4):
                        t = wp.tile([128, 2 * DI], bf, tag=f"inw{k}")
                        nc.sync.dma_start(t[:], din[f"inW_{d}"][128 * k:128 * (k + 1), :])
                        inw.append(t)

                    xs = []     # pre-conv x_ssm tiles (128, HALO) f32
                    for m in range(16):
                        px = ps1.tile([128, HALO], f32, tag="p1")
                        for k in range(4):
                            nc.tensor.matmul(px[:], inw[k][:, 128 * m:128 * (m + 1)],
                                             xsb[k][:], start=(k == 0), stop=(k == 3))
                        if m < 8:
                            t = ap_.tile([128, HALO], f32, tag=f"xs{m}")
                            nc.scalar.activation(t[:], px[:], Act.Identity,
                                                 bias=inbx[:, m:m + 1])
                            xs.append(t)
                        else:
                            zt = pp.tile([128, LC], bf, tag=f"z{d}{m - 8}")
                            nc.scalar.activation(zt[:], px[:, 3:3 + LC], Act.Silu,
                                                 bias=inbz[:, m - 8:m - 7])
                            zs[(d, m - 8)] = zt

                    # depthwise causal conv + silu
                    xconv = []
                    silu_x = []
                    for m in range(8):
                        a0 = ap_.tile([128, LC], f32, tag="cacc0")
                        nc.vector.tensor_scalar(a0[:], xs[m][:, off:off + LC],
                                                convw[:, 4 * m:4 * m + 1],
                                                convb[:, m:m + 1],
                                                Alu.mult, Alu.add)
                        a1 = ap_.tile([128, LC], f32, tag="cacc1")
                        nc.vector.scalar_tensor_tensor(a1[:], xs[m][:, off + 1:off + 1 + LC],
                                                       convw[:, 4 * m + 1:4 * m + 2], a0[:],
                                                       Alu.mult, Alu.add)
                        a2 = ap_.tile([128, LC], f32, tag="cacc2")
                        nc.vector.scalar_tensor_tensor(a2[:], xs[m][:, off + 2:off + 2 + LC],
                                                       convw[:, 4 * m + 2:4 * m + 3], a1[:],
                                                       Alu.mult, Alu.add)
                        xc = ap_.tile([128, LC], f32r, tag=f"xc{m}")
                        nc.vector.scalar_tensor_tensor(xc[:], xs[m][:, off + 3:off + 3 + LC],
                                                       convw[:, 4 * m + 3:4 * m + 4], a2[:],
                                                       Alu.mult, Alu.add)
                        xconv.append(xc)
                        sx = ap_.tile([128, LC], bf, tag=f"sx{m}")
                        nc.scalar.activation(sx[:], xc[:], Act.Silu)
                        silu_x.append(sx)
                        nc.sync.dma_start(c1_in[didx][m, 1, :, :], xc[:])

                    # x-proj
                    xpw = []
                    for k in range(8):
                        t = wp.tile([128, DI + 2 * D_STATE], bf, tag=f"xpw{k}")
                        nc.sync.dma_start(t[:], din[f"xpW_{d}"][128 * k:128 * (k + 1), :])
                        xpw.append(t)
                    delta = []
                    for m in range(9):
                        rows = 128 if m < 8 else 32
                        px = ps1.tile([128, LC], f32, tag="p1")
                        for k in range(8):
                            nc.tensor.matmul(px[:rows, :],
                                             xpw[k][:, 128 * m:128 * m + rows],
                                             silu_x[k][:], start=(k == 0), stop=(k == 7))
                        if m < 8:
                            t = ap_.tile([128, LC], bf, tag=f"dl{m}")
                            nc.scalar.activation(t[:], px[:], Act.Identity,
                                                 bias=xpbd[:, m:m + 1])
                            delta.append(t)
                        else:
                            bct = ap_.tile([32, LC], f32r, tag="bc")
                            nc.scalar.activation(bct[:], px[:32, :], Act.Identity,
                                                 bias=xpbbc[:])
                            nc.sync.dma_start(ag_in[didx][:, :], bct[:])

                    # dt-proj + softplus
                    dtw = []
                    for k in range(8):
                        t = wp.tile([128, DI], bf, tag=f"dtw{k}")
                        nc.sync.dma_start(t[:], din[f"dtW_{d}"][128 * k:128 * (k + 1), :])
                        dtw.append(t)
                    for m in range(8):
                        px = ps1.tile([128, LC], f32, tag="p1")
                        for k in range(8):
                            nc.tensor.matmul(px[:], dtw[k][:, 128 * m:128 * (m + 1)],
                                             delta[k][:], start=(k == 0), stop=(k == 7))
                        t = ap_.tile([128, LC], f32r, tag=f"dts{m}")
                        nc.scalar.activation(t[:], px[:], Act.Identity,
                                             bias=dtb[:, m:m + 1])
                        nc.sync.dma_start(c1_in[didx][m, 0, :, :], t[:])

            # ================= COLLECTIVES 1 (per direction) =================
            for i in range(2):
                nc.gpsimd.collective_compute(
                    "AllToAll", Alu.bypass, replica_groups=RG,
                    ins=[c1_in[i][:].opt()], outs=[c1_out[i][:].opt()])
                nc.gpsimd.collective_compute(
                    "AllGather", Alu.bypass, replica_groups=RG,
                    ins=[ag_in[i][:].opt()], outs=[ag_out[i][:].opt()])

            # ================= PHASE 2 =================
            for didx, d in enumerate(("f", "b")):
                rev = (d == "b")
                with tc.tile_pool(name=f"p2_{d}", bufs=1) as p2, \
                     tc.tile_pool(name=f"psA_{d}", bufs=2, space="PSUM") as psA, \
                     tc.tile_pool(name=f"psB_{d}", bufs=2, space="PSUM") as psB, \
                     tc.tile_pool(name=f"psY_{d}", bufs=1, space="PSUM") as psY:

                    xc_m = p2.tile([128, L], f32r, tag="xcm")
                    dt_m = p2.tile([128, L], f32r, tag="dtm")
                    dtx = p2.tile([128, L], f32r, tag="dtx")
                    brep = p2.tile([128, L], f32, tag="brep")
                    crep = p2.tile([128, L], bf, tag="crep")

                    with tc.tile_pool(name=f"p2s_{d}", bufs=1) as sc:
                        u_m = sc.tile([128, L], f32r, tag="um")
                        if rev:
                            u_d = sc.tile([128, L], f32r, tag="ud")
                            xc_d = sc.tile([128, L], f32r, tag="xd")
                            nc.sync.dma_start(
                                u_d[:].rearrange("p (s c) -> p s c", s=NCORES),
                                c1_out[didx][:, 0, :, :].rearrange("s p c -> p s c"))
                            nc.sync.dma_start(
                                xc_d[:].rearrange("p (s c) -> p s c", s=NCORES),
                                c1_out[didx][:, 1, :, :].rearrange("s p c -> p s c"))
                            nc.vector.tensor_copy(u_m[:], u_d[:, ::-1])
                            nc.vector.tensor_copy(xc_m[:], xc_d[:, ::-1])
                        else:
                            nc.sync.dma_start(
                                u_m[:].rearrange("p (s c) -> p s c", s=NCORES),
                                c1_out[didx][:, 0, :, :].rearrange("s p c -> p s c"))
                            nc.sync.dma_start(
                                xc_m[:].rearrange("p (s c) -> p s c", s=NCORES),
                                c1_out[didx][:, 1, :, :].rearrange("s p c -> p s c"))

                        # softplus(u) = silu(u) + P3(min(u^2, 2.25)), chunked
                        for q in range(NQ):
                            sl = slice(MQ * q, MQ * (q + 1))
                            su = sc.tile([128, MQ], f32, tag="sp_su")
                            nc.scalar.activation(su[:], u_m[:, sl], Act.Silu)
                            qq = sc.tile([128, MQ], f32, tag="sp_q")
                            nc.scalar.activation(qq[:], u_m[:, sl], Act.Square)
                            qc = sc.tile([128, MQ], f32, tag="sp_qc")
                            nc.vector.tensor_scalar_min(qc[:], qq[:], 2.25)
                            aa = sc.tile([128, MQ], f32, tag="sp_a")
                            nc.vector.tensor_scalar(aa[:], qc[:], SP_C[1], SP_C[0],
                                                    Alu.mult, Alu.add)
                            bb = sc.tile([128, MQ], f32, tag="sp_b")
                            nc.vector.tensor_scalar(bb[:], qc[:], SP_C[3], SP_C[2],
                                                    Alu.mult, Alu.add)
                            t2 = sc.tile([128, MQ], f32, tag="sp_t2")
                            nc.gpsimd.tensor_tensor(t2[:], qc[:], qc[:], Alu.mult)
                            m1 = sc.tile([128, MQ], f32, tag="sp_m1")
                            nc.gpsimd.tensor_tensor(m1[:], t2[:], bb[:], Alu.mult)
                            s1 = sc.tile([128, MQ], f32, tag="sp_s1")
                            nc.gpsimd.tensor_tensor(s1[:], aa[:], m1[:], Alu.add)
                            nc.vector.tensor_tensor(dt_m[:, sl], s1[:], su[:], Alu.add)
                        nc.vector.tensor_tensor(dtx[:], dt_m[:], xc_m[:], Alu.mult)

                        # broadcast B, C across the 16-sigma partition groups
                        B_m = sc.tile([16, L], f32r, tag="bm")
                        C_m = sc.tile([16, L], f32r, tag="cm")
                        if rev:
                            B_sb = sc.tile([16, L], f32r, tag="bsb")
                            C_sb = sc.tile([16, L], f32r, tag="csb")
                            nc.sync.dma_start(
                                B_sb[:].rearrange("p (s c) -> p s c", s=NCORES),
                                ag_out[didx][:, 0:16, :].rearrange("s p c -> p s c"))
                            nc.sync.dma_start(
                                C_sb[:].rearrange("p (s c) -> p s c", s=NCORES),
                                ag_out[didx][:, 16:32, :].rearrange("s p c -> p s c"))
                            nc.vector.tensor_copy(B_m[:], B_sb[:, ::-1])
                            nc.vector.tensor_copy(C_m[:], C_sb[:, ::-1])
                        else:
                            nc.sync.dma_start(
                                B_m[:].rearrange("p (s c) -> p s c", s=NCORES),
                                ag_out[didx][:, 0:16, :].rearrange("s p c -> p s c"))
                            nc.sync.dma_start(
                                C_m[:].rearrange("p (s c) -> p s c", s=NCORES),
                                ag_out[didx][:, 16:32, :].rearrange("s p c -> p s c"))
                        for q in range(NQ):
                            sl = slice(MQ * q, MQ * (q + 1))
                            pq = psA.tile([128, MQ], f32, tag="pa")
                            nc.tensor.matmul(pq[:], e16[:], B_m[:, sl],
                                             start=True, stop=True)
                            nc.scalar.activation(brep[:, sl], pq[:], Act.Copy)
                            pq2 = psA.tile([128, MQ], f32, tag="pa")
                            nc.tensor.matmul(pq2[:], e16[:], C_m[:, sl],
                                             start=True, stop=True)
                            nc.scalar.activation(crep[:, sl], pq2[:], Act.Copy)

                    ypsum = psY.tile([128, L], f32, tag="ypsum")
                    with tc.tile_pool(name=f"p2t_{d}", bufs=2) as tp:
                        for t in range(NT):
                            dA = tp.tile([128, L], f32, tag="dA")
                            dBu = tp.tile([128, L], f32, tag="dBu")
                            for q in range(NQ):
                                sl = slice(MQ * q, MQ * (q + 1))
                                pa = psA.tile([128, MQ], f32, tag="pa")
                                nc.tensor.matmul(pa[:], e128[:, 128 * t:128 * (t + 1)],
                                                 dt_m[:, sl], start=True, stop=True)
                                nc.scalar.activation(dA[:, sl], pa[:], Act.Exp,
                                                     scale=alan[:, t:t + 1])
                                pb = psB.tile([128, MQ], f32, tag="pb")
                                nc.tensor.matmul(pb[:], e128[:, 128 * t:128 * (t + 1)],
                                                 dtx[:, sl], start=True, stop=True)
                                if t % 3 == 0:
                                    nc.vector.tensor_tensor(dBu[:, sl], pb[:],
                                                            brep[:, sl], Alu.mult)
                                else:
                                    dxr = tp.tile([128, MQ], f32, tag="dxr")
                                    nc.scalar.activation(dxr[:], pb[:], Act.Copy)
                                    nc.gpsimd.tensor_tensor(dBu[:, sl], dxr[:],
                                                            brep[:, sl], Alu.mult)
                            h = tp.tile([128, L], bf, tag="h")
                            nc.vector.tensor_tensor_scan(h[:], dA[:], dBu[:], 0.0,
                                                         Alu.mult, Alu.add)
                            yp = tp.tile([128, L], bf, tag="yp")
                            yeng = nc.vector if (t % 2 == 0) else nc.gpsimd
                            yeng.tensor_tensor(yp[:], h[:], crep[:], Alu.mult)
                            for q in range(NQ):
                                sl = slice(MQ * q, MQ * (q + 1))
                                nc.tensor.matmul(ypsum[:, sl],
                                                 sel128[:, 128 * t:128 * (t + 1)],
                                                 yp[:, sl],
                                                 start=(t == 0), stop=(t == NT - 1),
                                                 skip_group_check=True)

                    y_sb = p2.tile([128, L], bf, tag="ysb")
                    nc.vector.scalar_tensor_tensor(y_sb[:], xc_m[:], dpl[:], ypsum[:],
                                                   Alu.mult, Alu.add)
                    if rev:
                        y_r = p2.tile([128, L], bf, tag="yr")
                        nc.vector.tensor_copy(y_r[:], y_sb[:, ::-1])
                    else:
                        y_r = y_sb
                    for dst in range(NCORES):
                        nc.sync.dma_start(c2_in[didx][dst, 0, :, :],
                                          y_r[:, LC * dst:LC * (dst + 1)])
                nc.gpsimd.collective_compute(
                    "AllToAll", Alu.bypass, replica_groups=RG,
                    ins=[c2_in[didx][:].opt()], outs=[c2_out[didx][:].opt()])

            # ================= PHASE 3 =================
            cat = []
            with tc.tile_pool(name="p3", bufs=2) as p3, \
                 tc.tile_pool(name="p3c", bufs=1) as p3c, \
                 tc.tile_pool(name="p3ps", bufs=4, space="PSUM") as ps3:
                for didx, d in enumerate(("f", "b")):
                    outb = p3w[("outb", d)]
                    gates = []
                    for m in range(8):
                        y3 = p3.tile([128, LC], bf, tag=f"y3{m}")
                        nc.sync.dma_start(y3[:], c2_out[didx][m, 0, :, :])
                        g = p3.tile([128, LC], bf, tag=f"g{m}")
                        nc.vector.tensor_tensor(g[:], y3[:], zs[(d, m)][:], Alu.mult)
                        gates.append(g)
                    outw = [p3w[("outw", d, k)] for k in range(8)]
                    for m in range(4):
                        po = ps3.tile([128, LC], f32, tag="p3a")
                        for k in range(8):
                            nc.tensor.matmul(po[:], outw[k][:, 128 * m:128 * (m + 1)],
                                             gates[k][:], start=(k == 0), stop=(k == 7))
                        ct = p3c.tile([128, LC], bf, tag=f"cat{didx}{m}")
                        nc.scalar.activation(ct[:], po[:], Act.Identity,
                                             bias=outb[:, m:m + 1])
                        cat.append(ct)
                # fusion
                fb = fbt
                fw = fwt
                for m in range(4):
                    pf = ps3.tile([128, LC], f32, tag="p3b")
                    for k in range(8):
                        nc.tensor.matmul(pf[:], fw[k][:, 128 * m:128 * (m + 1)],
                                         cat[k][:], start=(k == 0), stop=(k == 7))
                    ot = p3.tile([128, LC], f32, tag="ot")
                    nc.scalar.activation(ot[:], pf[:], Act.Identity, bias=fb[:, m:m + 1])
                    nc.sync.dma_start(outT[128 * m:128 * (m + 1), :], ot[:])

    nc.compile()
    return nc


def make_in_maps(inputs):
    x = np.asarray(inputs["x"], np.float32)
    A = -np.exp(np.asarray(inputs["A_log"], np.float32))          # (DI, S)
    Dp = np.asarray(inputs["D_param"], np.float32)

    def bias_tiles(b, ntiles):
        return np.ascontiguousarray(
            np.asarray(b, np.float32).reshape(ntiles, 128).T)

    common = {}
    for d, pre in (("f", "fwd_"), ("b", "bwd_")):
        inW = np.asarray(inputs[pre + "in_W"], np.float32)
        inb = np.asarray(inputs[pre + "in_b"], np.float32)
        cw = np.asarray(inputs[pre + "conv_w"], np.float32)
        if d == "b":
            cw = cw[:, ::-1]
        cb = np.asarray(inputs[pre + "conv_b"], np.float32)
        xpW = np.asarray(inputs[pre + "xp_W"], np.float32)
        xpb = np.asarray(inputs[pre + "xp_b"], np.float32)
        dtW = np.asarray(inputs[pre + "dt_W"], np.float32)
        dtb = np.asarray(inputs[pre + "dt_b"], np.float32)
        outW = np.asarray(inputs[pre + "out_W"], np.float32)
        outb = np.asarray(inputs[pre + "out_b"], np.float32)
        common[f"inW_{d}"] = inW.astype(BF16)
        common[f"inbx_{d}"] = bias_tiles(inb[:DI], 8)
        common[f"inbz_{d}"] = bias_tiles(inb[DI:], 8)
        common[f"convw_{d}"] = np.ascontiguousarray(
            cw.reshape(8, 128, 4).transpose(1, 0, 2).reshape(128, 32))
        common[f"convb_{d}"] = bias_tiles(cb, 8)
        common[f"xpW_{d}"] = xpW.astype(BF16)
        common[f"xpbd_{d}"] = bias_tiles(xpb[:DI], 8)
        common[f"xpbbc_{d}"] = np.ascontiguousarray(xpb[DI:].reshape(32, 1))
        common[f"dtW_{d}"] = dtW.astype(BF16)
        common[f"dtb_{d}"] = bias_tiles(dtb, 8)
        common[f"outW_{d}"] = outW.astype(BF16)
        common[f"outb_{d}"] = bias_tiles(outb, 4)
    common["fusW"] = np.asarray(inputs["fusion_W"], np.float32).astype(BF16)
    common["fusb"] = bias_tiles(np.asarray(inputs["fusion_b"], np.float32), 4)

    p = np.arange(128)
    e128 = np.zeros((128, 16 * 128), np.float32)
    sel128 = np.zeros((128, 16 * 128), np.float32)
    for t in range(16):
        e128[8 * t + p // 16, 128 * t + p] = 1.0
        sel128[p, 128 * t + 8 * t + p // 16] = 1.0
    e16 = np.zeros((16, 128), np.float32)
    e16[p % 16, p] = 1.0
    common["E128m"] = e128
    common["E16m"] = e16
    common["SEL128m"] = sel128.astype(BF16)

    in_maps = []
    for c in range(NCORES):
        m = dict(common)
        r0 = LC * c
        xpad = np.zeros((HALO, D_MODEL), np.float32)
        lo, hi = max(0, r0 - 3), min(L, r0 + LC + 3)
        xpad[lo - (r0 - 3): hi - (r0 - 3)] = x[lo:hi]
        m["xT"] = np.ascontiguousarray(xpad.T).astype(BF16)
        A_sh = A[128 * c:128 * (c + 1)]                      # (128, 16)
        m["Alan"] = np.ascontiguousarray(
            A_sh.reshape(16, 8, 16).transpose(1, 2, 0).reshape(128, NT))
        m["Dpl"] = np.ascontiguousarray(Dp[128 * c:128 * (c + 1)].reshape(128, 1))
        in_maps.append(m)
    return in_maps


_CACHE = {}


def kernel(**inputs):
    from concourse.bass_utils import run_bass_kernel_spmd
    if "nc" not in _CACHE:
        _CACHE["nc"] = build_bass()
    nc = _CACHE["nc"]
    in_maps = make_in_maps(inputs)
    res = run_bass_kernel_spmd(nc, in_maps, list(range(NCORES)))
    outs = [res.results[c]["outT"] for c in range(NCORES)]
    full = np.concatenate(outs, axis=1)      # (512, 2048)
    return np.ascontiguousarray(full.T).astype(np.float32)


# revision 14
# speedup vs baseline: 1.1970x; 1.1225x over previous
"""Bidirectional selective-scan SSM (CausalMolSSM) on 8 TRN2 NeuronCores.

Strategy:
  Phase 1 (L-sharded): each core owns 256 sequence rows (+conv halo) and
    computes, in transposed feature-major layout, the in-proj, depthwise
    causal conv, silu, x-proj, dt-proj for both directions.
  AllToAll #1 + AllGather: redistribute dt / x_conv to channel-sharded
    layout (128 channels/core, full L=2048), broadcast B/C.
  Phase 2 (channel-sharded): build dA=exp(dt*A) and dBu=dt*x*B on lanes
    (d,sigma), run the linear recurrence with tensor_tensor_scan along L,
    contract over sigma with a selection matmul, add D*x term.
  AllToAll #2: bring y back to L-sharded layout.
  Phase 3 (L-sharded): gate with silu(z), out-proj, concat, fusion proj.
  The backward direction is handled by the same cores via index reversal
  (core c owns reversed-chunk 7-c, which uses the same x rows as chunk c).
"""
import sys
sys.path.insert(0, '/opt/trn_rl_repo')
import numpy as np
import ml_dtypes

D_MODEL, D_STATE, D_CONV, L = 512, 16, 4, 2048
DI = 1024
NCORES = 8
LC = L // NCORES            # 256
HALO = LC + 6               # 262
DSH = DI // NCORES          # 128 channels per core
NT = DSH * D_STATE // 128   # 16 lane tiles per direction
MQ = 512                    # phase-2 free-dim chunk (one PSUM bank)
NQ = L // MQ                # 4

BF16 = ml_dtypes.bfloat16

# softplus(u) = silu(u) + P(min(u^2, 2.25)); even-part Chebyshev fit, |u|<=1.5
SP_C = (0.6931054730054999, -0.12461714435972031,
        0.014817950932472132, -0.0011165576908336256)

# engine assignment for the per-lane-tile y multiply (tunable)
YMUL_GPSIMD_MOD = 2   # t % YMUL_GPSIMD_MOD == 1 -> gpsimd


def build_bass():
    import concourse.bass as bass
    import concourse.bacc as bacc
    import concourse.tile as tile
    import concourse.mybir as mybir

    dt = mybir.dt
    Alu = mybir.AluOpType
    Act = mybir.ActivationFunctionType

    nc = bacc.Bacc("TRN2", target_bir_lowering=False, debug=False,
                   enable_asserts=True, num_devices=NCORES)

    f32, f32r, bf = dt.float32, dt.float32r, dt.bfloat16

    # ---------------- DRAM I/O ----------------
    xT = nc.dram_tensor("xT", [D_MODEL, HALO], bf, kind="ExternalInput")
    din = {}
    for d in ("f", "b"):
        din[f"inW_{d}"] = nc.dram_tensor(f"inW_{d}", [D_MODEL, 2 * DI], bf, kind="ExternalInput")
        din[f"xpW_{d}"] = nc.dram_tensor(f"xpW_{d}", [DI, DI + 2 * D_STATE], bf, kind="ExternalInput")
        din[f"dtW_{d}"] = nc.dram_tensor(f"dtW_{d}", [DI, DI], bf, kind="ExternalInput")
        din[f"outW_{d}"] = nc.dram_tensor(f"outW_{d}", [DI, D_MODEL], bf, kind="ExternalInput")
        din[f"inbx_{d}"] = nc.dram_tensor(f"inbx_{d}", [128, 8], f32, kind="ExternalInput")
        din[f"inbz_{d}"] = nc.dram_tensor(f"inbz_{d}", [128, 8], f32, kind="ExternalInput")
        din[f"xpbd_{d}"] = nc.dram_tensor(f"xpbd_{d}", [128, 8], f32, kind="ExternalInput")
        din[f"xpbbc_{d}"] = nc.dram_tensor(f"xpbbc_{d}", [32, 1], f32, kind="ExternalInput")
        din[f"dtb_{d}"] = nc.dram_tensor(f"dtb_{d}", [128, 8], f32, kind="ExternalInput")
        din[f"outb_{d}"] = nc.dram_tensor(f"outb_{d}", [128, 4], f32, kind="ExternalInput")
        din[f"convw_{d}"] = nc.dram_tensor(f"convw_{d}", [128, 32], f32, kind="ExternalInput")
        din[f"convb_{d}"] = nc.dram_tensor(f"convb_{d}", [128, 8], f32, kind="ExternalInput")
    fusW = nc.dram_tensor("fusW", [2 * D_MODEL, D_MODEL], bf, kind="ExternalInput")
    fusb = nc.dram_tensor("fusb", [128, 4], f32, kind="ExternalInput")
    Alan = nc.dram_tensor("Alan", [128, NT], f32, kind="ExternalInput")
    Dpl = nc.dram_tensor("Dpl", [128, 1], f32, kind="ExternalInput")
    E128m = nc.dram_tensor("E128m", [128, 16 * 128], f32r, kind="ExternalInput")
    E16m = nc.dram_tensor("E16m", [16, 128], f32r, kind="ExternalInput")
    SEL128m = nc.dram_tensor("SEL128m", [128, 16 * 128], bf, kind="ExternalInput")
    outT = nc.dram_tensor("outT", [D_MODEL, LC], f32, kind="ExternalOutput")

    RG = [list(range(NCORES))]

    with tile.TileContext(nc) as tc:
        with tc.tile_pool(name="dram", bufs=1, space="DRAM") as dram, \
             tc.tile_pool(name="persist", bufs=1) as pp, \
             tc.tile_pool(name="const", bufs=1) as cp:

            c1_in = [dram.tile([NCORES, 2, 128, LC], f32r, tag=f"c1in{i}", name=f"c1in{i}")
                     for i in range(2)]
            c1_out = [dram.tile([NCORES, 2, 128, LC], f32r, tag=f"c1out{i}", name=f"c1out{i}")
                      for i in range(2)]
            ag_in = [dram.tile([32, LC], f32r, tag=f"agin{i}", name=f"agin{i}") for i in range(2)]
            ag_out = [dram.tile([NCORES, 32, LC], f32r, tag=f"agout{i}", name=f"agout{i}")
                      for i in range(2)]
            c2_in = [dram.tile([NCORES, 1, 128, LC], bf, tag=f"c2in{i}", name=f"c2in{i}")
                     for i in range(2)]
            c2_out = [dram.tile([NCORES, 1, 128, LC], bf, tag=f"c2out{i}", name=f"c2out{i}")
                      for i in range(2)]

            # constants
            e128 = cp.tile([128, 16 * 128], f32r, tag="e128")
            e16 = cp.tile([16, 128], f32r, tag="e16")
            sel128 = cp.tile([128, 16 * 128], bf, tag="sel128")
            alan = cp.tile([128, NT], f32, tag="alan")
            dpl = cp.tile([128, 1], f32, tag="dpl")
            nc.sync.dma_start(e128[:], E128m[:])
            nc.sync.dma_start(e16[:], E16m[:])
            nc.sync.dma_start(sel128[:], SEL128m[:])
            nc.sync.dma_start(alan[:], Alan[:])
            nc.sync.dma_start(dpl[:], Dpl[:])

            zs = {}   # persistent silu(z) tiles, (128, LC) bf16, [dir][m]

            # prefetch phase-3 weights early (DMA overlaps phases 1-2)
            p3w = {}
            for didx, d in enumerate(("f", "b")):
                ob = pp.tile([128, 4], f32, tag=f"outb{d}")
                nc.sync.dma_start(ob[:], din[f"outb_{d}"][:])
                p3w[("outb", d)] = ob
                for k in range(8):
                    t = pp.tile([128, D_MODEL], bf, tag=f"outw{d}{k}")
                    nc.sync.dma_start(t[:], din[f"outW_{d}"][128 * k:128 * (k + 1), :])
                    p3w[("outw", d, k)] = t
            fbt = pp.tile([128, 4], f32, tag="fusb")
            nc.sync.dma_start(fbt[:], fusb[:])
            fwt = []
            for k in range(8):
                t = pp.tile([128, D_MODEL], bf, tag=f"fw{k}")
                nc.sync.dma_start(t[:], fusW[128 * k:128 * (k + 1), :])
                fwt.append(t)

            # ================= PHASE 1 =================
            for didx, d in enumerate(("f", "b")):
                off = 0 if d == "f" else 3
                with tc.tile_pool(name=f"p1w_{d}", bufs=1) as wp, \
                     tc.tile_pool(name=f"p1a_{d}", bufs=1) as ap_, \
                     tc.tile_pool(name=f"p1ps_{d}", bufs=4, space="PSUM") as ps1, \
                     tc.tile_pool(name=f"p1sc_{d}", bufs=3) as scp:

                    # biases
                    inbx = scp.tile([128, 8], f32, tag="inbx")
                    inbz = scp.tile([128, 8], f32, tag="inbz")
                    xpbd = scp.tile([128, 8], f32, tag="xpbd")
                    xpbbc = scp.tile([32, 1], f32, tag="xpbbc")
                    dtb = scp.tile([128, 8], f32, tag="dtb")
                    convw = scp.tile([128, 32], f32, tag="convw")
                    convb = scp.tile([128, 8], f32, tag="convb")
                    nc.sync.dma_start(inbx[:], din[f"inbx_{d}"][:])
                    nc.sync.dma_start(inbz[:], din[f"inbz_{d}"][:])
                    nc.sync.dma_start(xpbd[:], din[f"xpbd_{d}"][:])
                    nc.sync.dma_start(xpbbc[:], din[f"xpbbc_{d}"][:])
                    nc.sync.dma_start(dtb[:], din[f"dtb_{d}"][:])
                    nc.sync.dma_start(convw[:], din[f"convw_{d}"][:])
                    nc.sync.dma_start(convb[:], din[f"convb_{d}"][:])

                    # x tiles
                    xsb = []
                    for k in range(4):
                        t = ap_.tile([128, HALO], bf, tag=f"x{k}")
                        nc.sync.dma_start(t[:], xT[128 * k:128 * (k + 1), :])
                        xsb.append(t)

                    # in-proj weights
                    inw = []
                    for k in range(4):
                        t = wp.tile([128, 2 * DI], bf, tag=f"inw{k}")
                        nc.sync.dma_start(t[:], din[f"inW_{d}"][128 * k:128 * (k + 1), :])
                        inw.append(t)

                    xs = []     # pre-conv x_ssm tiles (128, HALO) f32
                    for m in range(16):
                        px = ps1.tile([128, HALO], f32, tag="p1")
                        for k in range(4):
                            nc.tensor.matmul(px[:], inw[k][:, 128 * m:128 * (m + 1)],
                                             xsb[k][:], start=(k == 0), stop=(k == 3))
                        if m < 8:
                            t = ap_.tile([128, HALO], f32, tag=f"xs{m}")
                            nc.scalar.activation(t[:], px[:], Act.Identity,
                                                 bias=inbx[:, m:m + 1])
                            xs.append(t)
                        else:
                            zt = pp.tile([128, LC], bf, tag=f"z{d}{m - 8}")
                            nc.scalar.activation(zt[:], px[:, 3:3 + LC], Act.Silu,
                                                 bias=inbz[:, m - 8:m - 7])
                            zs[(d, m - 8)] = zt

                    # depthwise causal conv + silu
                    xconv = []
                    silu_x = []
                    for m in range(8):
                        a0 = ap_.tile([128, LC], f32, tag="cacc0")
                        nc.vector.tensor_scalar(a0[:], xs[m][:, off:off + LC],
                                                convw[:, 4 * m:4 * m + 1],
                                                convb[:, m:m + 1],
                                                Alu.mult, Alu.add)
                        a1 = ap_.tile([128, LC], f32, tag="cacc1")
                        nc.vector.scalar_tensor_tensor(a1[:], xs[m][:, off + 1:off + 1 + LC],
                                                       convw[:, 4 * m + 1:4 * m + 2], a0[:],
                                                       Alu.mult, Alu.add)
                        a2 = ap_.tile([128, LC], f32, tag="cacc2")
                        nc.vector.scalar_tensor_tensor(a2[:], xs[m][:, off + 2:off + 2 + LC],
                                                       convw[:, 4 * m + 2:4 * m + 3], a1[:],
                                                       Alu.mult, Alu.add)
                        xc = ap_.tile([128, LC], f32r, tag=f"xc{m}")
                        nc.vector.scalar_tensor_tensor(xc[:], xs[m][:, off + 3:off + 3 + LC],
                                                       convw[:, 4 * m + 3:4 * m + 4], a2[:],
                                                       Alu.mult, Alu.add)
                        xconv.append(xc)
                        sx = ap_.tile([128, LC], bf, tag=f"sx{m}")
                        nc.scalar.activation(sx[:], xc[:], Act.Silu)
                        silu_x.append(sx)
                        nc.sync.dma_start(c1_in[didx][m, 1, :, :], xc[:])

                    # x-proj
                    xpw = []
                    for k in range(8):
                        t = wp.tile([128, DI + 2 * D_STATE], bf, tag=f"xpw{k}")
                        nc.sync.dma_start(t[:], din[f"xpW_{d}"][128 * k:128 * (k + 1), :])
                        xpw.append(t)
                    delta = []
                    for m in range(9):
                        rows = 128 if m < 8 else 32
                        px = ps1.tile([128, LC], f32, tag="p1")
                        for k in range(8):
                            nc.tensor.matmul(px[:rows, :],
                                             xpw[k][:, 128 * m:128 * m + rows],
                                             silu_x[k][:], start=(k == 0), stop=(k == 7))
                        if m < 8:
                            t = ap_.tile([128, LC], bf, tag=f"dl{m}")
                            nc.scalar.activation(t[:], px[:], Act.Identity,
                                                 bias=xpbd[:, m:m + 1])
                            delta.append(t)
                        else:
                            bct = ap_.tile([32, LC], f32r, tag="bc")
                            nc.scalar.activation(bct[:], px[:32, :], Act.Identity,
                                                 bias=xpbbc[:])
                            nc.sync.dma_start(ag_in[didx][:, :], bct[:])

                    # dt-proj + softplus
                    dtw = []
                    for k in range(8):
                        t = wp.tile([128, DI], bf, tag=f"dtw{k}")
                        nc.sync.dma_start(t[:], din[f"dtW_{d}"][128 * k:128 * (k + 1), :])
                        dtw.append(t)
                    for m in range(8):
                        px = ps1.tile([128, LC], f32, tag="p1")
                        for k in range(8):
                            nc.tensor.matmul(px[:], dtw[k][:, 128 * m:128 * (m + 1)],
                                             delta[k][:], start=(k == 0), stop=(k == 7))
                        t = ap_.tile([128, LC], f32r, tag=f"dts{m}")
                        nc.scalar.activation(t[:], px[:], Act.Identity,
                                             bias=dtb[:, m:m + 1])
                        nc.sync.dma_start(c1_in[didx][m, 0, :, :], t[:])

            # ================= COLLECTIVES 1 (per direction) =================
            for i in range(2):
                nc.gpsimd.collective_compute(
                    "AllToAll", Alu.bypass, replica_groups=RG,
                    ins=[c1_in[i][:].opt()], outs=[c1_out[i][:].opt()])
                nc.gpsimd.collective_compute(
                    "AllGather", Alu.bypass, replica_groups=RG,
                    ins=[ag_in[i][:].opt()], outs=[ag_out[i][:].opt()])

            # ================= PHASE 2 =================
            for didx, d in enumerate(("f", "b")):
                rev = (d == "b")
                with tc.tile_pool(name=f"p2_{d}", bufs=1) as p2, \
                     tc.tile_pool(name=f"psA_{d}", bufs=2, space="PSUM") as psA, \
                     tc.tile_pool(name=f"psB_{d}", bufs=2, space="PSUM") as psB, \
                     tc.tile_pool(name=f"psY_{d}", bufs=1, space="PSUM") as psY:

                    xc_m = p2.tile([128, L], f32r, tag="xcm")
                    dt_m = p2.tile([128, L], f32r, tag="dtm")
                    dtx = p2.tile([128, L], f32r, tag="dtx")
                    brep = p2.tile([128, L], f32, tag="brep")
                    crep = p2.tile([128, L], bf, tag="crep")

                    with tc.tile_pool(name=f"p2s_{d}", bufs=1) as sc:
                        u_m = sc.tile([128, L], f32r, tag="um")
                        if rev:
                            u_d = sc.tile([128, L], f32r, tag="ud")
                            xc_d = sc.tile([128, L], f32r, tag="xd")
                            nc.sync.dma_start(
                                u_d[:].rearrange("p (s c) -> p s c", s=NCORES),
                                c1_out[didx][:, 0, :, :].rearrange("s p c -> p s c"))
                            nc.sync.dma_start(
                                xc_d[:].rearrange("p (s c) -> p s c", s=NCORES),
                                c1_out[didx][:, 1, :, :].rearrange("s p c -> p s c"))
                            nc.vector.tensor_copy(u_m[:], u_d[:, ::-1])
                            nc.vector.tensor_copy(xc_m[:], xc_d[:, ::-1])
                        else:
                            nc.sync.dma_start(
                                u_m[:].rearrange("p (s c) -> p s c", s=NCORES),
                                c1_out[didx][:, 0, :, :].rearrange("s p c -> p s c"))
                            nc.sync.dma_start(
                                xc_m[:].rearrange("p (s c) -> p s c", s=NCORES),
                                c1_out[didx][:, 1, :, :].rearrange("s p c -> p s c"))

                        # softplus(u) = silu(u) + P3(min(u^2, 2.25)), chunked
                        for q in range(NQ):
                            sl = slice(MQ * q, MQ * (q + 1))
                            su = sc.tile([128, MQ], f32, tag="sp_su")
                            nc.scalar.activation(su[:], u_m[:, sl], Act.Silu)
                            qq = sc.tile([128, MQ], f32, tag="sp_q")
                            nc.scalar.activation(qq[:], u_m[:, sl], Act.Square)
                            qc = sc.tile([128, MQ], f32, tag="sp_qc")
                            nc.vector.tensor_scalar_min(qc[:], qq[:], 2.25)
                            aa = sc.tile([128, MQ], f32, tag="sp_a")
                            nc.vector.tensor_scalar(aa[:], qc[:], SP_C[1], SP_C[0],
                                                    Alu.mult, Alu.add)
                            bb = sc.tile([128, MQ], f32, tag="sp_b")
                            nc.vector.tensor_scalar(bb[:], qc[:], SP_C[3], SP_C[2],
                                                    Alu.mult, Alu.add)
                            t2 = sc.tile([128, MQ], f32, tag="sp_t2")
                            nc.gpsimd.tensor_tensor(t2[:], qc[:], qc[:], Alu.mult)
                            m1 = sc.tile([128, MQ], f32, tag="sp_m1")
                            nc.gpsimd.tensor_tensor(m1[:], t2[:], bb[:], Alu.mult)
                            s1 = sc.tile([128, MQ], f32, tag="sp_s1")
                            nc.gpsimd.tensor_tensor(s1[:], aa[:], m1[:], Alu.add)
                            nc.vector.tensor_tensor(dt_m[:, sl], s1[:], su[:], Alu.add)
                        nc.vector.tensor_tensor(dtx[:], dt_m[:], xc_m[:], Alu.mult)

                        # broadcast B, C across the 16-sigma partition groups
                        B_m = sc.tile([16, L], f32r, tag="bm")
                        C_m = sc.tile([16, L], f32r, tag="cm")
                        if rev:
                            B_sb = sc.tile([16, L], f32r, tag="bsb")
                            C_sb = sc.tile([16, L], f32r, tag="csb")
                            nc.sync.dma_start(
                                B_sb[:].rearrange("p (s c) -> p s c", s=NCORES),
                                ag_out[didx][:, 0:16, :].rearrange("s p c -> p s c"))
                            nc.sync.dma_start(
                                C_sb[:].rearrange("p (s c) -> p s c", s=NCORES),
                                ag_out[didx][:, 16:32, :].rearrange("s p c -> p s c"))
                            nc.vector.tensor_copy(B_m[:], B_sb[:, ::-1])
                            nc.vector.tensor_copy(C_m[:], C_sb[:, ::-1])
                        else:
                            nc.sync.dma_start(
                                B_m[:].rearrange("p (s c) -> p s c", s=NCORES),
                                ag_out[didx][:, 0:16, :].rearrange("s p c -> p s c"))
                            nc.sync.dma_start(
                                C_m[:].rearrange("p (s c) -> p s c", s=NCORES),
                                ag_out[didx][:, 16:32, :].rearrange("s p c -> p s c"))
                        for q in range(NQ):
                            sl = slice(MQ * q, MQ * (q + 1))
                            pq = psA.tile([128, MQ], f32, tag="pa")
                            nc.tensor.matmul(pq[:], e16[:], B_m[:, sl],
                                             start=True, stop=True)
                            nc.scalar.activation(brep[:, sl], pq[:], Act.Copy)
                            pq2 = psA.tile([128, MQ], f32, tag="pa")
                            nc.tensor.matmul(pq2[:], e16[:], C_m[:, sl],
                                             start=True, stop=True)
                            nc.scalar.activation(crep[:, sl], pq2[:], Act.Copy)

                    ypsum = psY.tile([128, L], f32, tag="ypsum")
                    with tc.tile_pool(name=f"p2t_{d}", bufs=2) as tp:
                        for t in range(NT):
                            dA = tp.tile([128, L], f32, tag="dA")
                            dBu = tp.tile([128, L], f32, tag="dBu")
                            for q in range(NQ):
                                sl = slice(MQ * q, MQ * (q + 1))
                                pa = psA.tile([128, MQ], f32, tag="pa")
                                nc.tensor.matmul(pa[:], e128[:, 128 * t:128 * (t + 1)],
                                                 dt_m[:, sl], start=True, stop=True)
                                nc.scalar.activation(dA[:, sl], pa[:], Act.Exp,
                                                     scale=alan[:, t:t + 1])
                                pb = psB.tile([128, MQ], f32, tag="pb")
                                nc.tensor.matmul(pb[:], e128[:, 128 * t:128 * (t + 1)],
                                                 dtx[:, sl], start=True, stop=True)
                                nc.vector.tensor_tensor(dBu[:, sl], pb[:], brep[:, sl],
                                                        Alu.mult)
                            h = tp.tile([128, L], bf, tag="h")
                            nc.vector.tensor_tensor_scan(h[:], dA[:], dBu[:], 0.0,
                                                         Alu.mult, Alu.add)
                            yp = tp.tile([128, L], bf, tag="yp")
                            nc.gpsimd.tensor_tensor(yp[:], h[:], crep[:], Alu.mult)
                            for q in range(NQ):
                                sl = slice(MQ * q, MQ * (q + 1))
                                nc.tensor.matmul(ypsum[:, sl],
                                                 sel128[:, 128 * t:128 * (t + 1)],
                                                 yp[:, sl],
                                                 start=(t == 0), stop=(t == NT - 1),
                                                 skip_group_check=True)

                    y_sb = p2.tile([128, L], bf, tag="ysb")
                    nc.vector.scalar_tensor_tensor(y_sb[:], xc_m[:], dpl[:], ypsum[:],
                                                   Alu.mult, Alu.add)
                    if rev:
                        y_r = p2.tile([128, L], bf, tag="yr")
                        nc.vector.tensor_copy(y_r[:], y_sb[:, ::-1])
                    else:
                        y_r = y_sb
                    for dst in range(NCORES):
                        nc.sync.dma_start(c2_in[didx][dst, 0, :, :],
                                          y_r[:, LC * dst:LC * (dst + 1)])
                nc.gpsimd.collective_compute(
                    "AllToAll", Alu.bypass, replica_groups=RG,
                    ins=[c2_in[didx][:].opt()], outs=[c2_out[didx][:].opt()])

            # ================= PHASE 3 =================
            cat = []
            with tc.tile_pool(name="p3", bufs=2) as p3, \
                 tc.tile_pool(name="p3c", bufs=1) as p3c, \
                 tc.tile_pool(name="p3ps", bufs=4, space="PSUM") as ps3:
                for didx, d in enumerate(("f", "b")):
                    outb = p3w[("outb", d)]
                    gates = []
                    for m in range(8):
                        y3 = p3.tile([128, LC], bf, tag=f"y3{m}")
                        nc.sync.dma_start(y3[:], c2_out[didx][m, 0, :, :])
                        g = p3.tile([128, LC], bf, tag=f"g{m}")
                        nc.vector.tensor_tensor(g[:], y3[:], zs[(d, m)][:], Alu.mult)
                        gates.append(g)
                    outw = [p3w[("outw", d, k)] for k in range(8)]
                    for m in range(4):
                        po = ps3.tile([128, LC], f32, tag="p3a")
                        for k in range(8):
                            nc.tensor.matmul(po[:], outw[k][:, 128 * m:128 * (m + 1)],
                                             gates[k][:], start=(k == 0), stop=(k == 7))
                        ct = p3c.tile([128, LC], bf, tag=f"cat{didx}{m}")
                        nc.scalar.activation(ct[:], po[:], Act.Identity,
                                             bias=outb[:, m:m + 1])
                        cat.append(ct)
                # fusion
                fb = fbt
                fw = fwt
                for m in range(4):
                    pf = ps3.tile([128, LC], f32, tag="p3b")
                    for k in range(8):
                        nc.tensor.matmul(pf[:], fw[k][:, 128 * m:128 * (m + 1)],
                                         cat[k][:], start=(k == 0), stop=(k == 7))
                    ot = p3.tile([128, LC], f32, tag="ot")
                    nc.scalar.activation(ot[:], pf[:], Act.Identity, bias=fb[:, m:m + 1])
                    nc.sync.dma_start(outT[128 * m:128 * (m + 1), :], ot[:])

    nc.compile()
    return nc


def make_in_maps(inputs):
    x = np.asarray(inputs["x"], np.float32)
    A = -np.exp(np.asarray(inputs["A_log"], np.float32))          # (DI, S)
    Dp = np.asarray(inputs["D_param"], np.float32)

    def bias_tiles(b, ntiles):
        return np.ascontiguousarray(
            np.asarray(b, np.float32).reshape(ntiles, 128).T)

    common = {}
    for d, pre in (("f", "fwd_"), ("b", "bwd_")):
        inW = np.asarray(inputs[pre + "in_W"], np.float32)
        inb = np.asarray(inputs[pre + "in_b"], np.float32)
        cw = np.asarray(inputs[pre + "conv_w"], np.float32)
        if d == "b":
            cw = cw[:, ::-1]
        cb = np.asarray(inputs[pre + "conv_b"], np.float32)
        xpW = np.asarray(inputs[pre + "xp_W"], np.float32)
        xpb = np.asarray(inputs[pre + "xp_b"], np.float32)
        dtW = np.asarray(inputs[pre + "dt_W"], np.float32)
        dtb = np.asarray(inputs[pre + "dt_b"], np.float32)
        outW = np.asarray(inputs[pre + "out_W"], np.float32)
        outb = np.asarray(inputs[pre + "out_b"], np.float32)
        common[f"inW_{d}"] = inW.astype(BF16)
        common[f"inbx_{d}"] = bias_tiles(inb[:DI], 8)
        common[f"inbz_{d}"] = bias_tiles(inb[DI:], 8)
        common[f"convw_{d}"] = np.ascontiguousarray(
            cw.reshape(8, 128, 4).transpose(1, 0, 2).reshape(128, 32))
        common[f"convb_{d}"] = bias_tiles(cb, 8)
        common[f"xpW_{d}"] = xpW.astype(BF16)
        common[f"xpbd_{d}"] = bias_tiles(xpb[:DI], 8)
        common[f"xpbbc_{d}"] = np.ascontiguousarray(xpb[DI:].reshape(32, 1))
        common[f"dtW_{d}"] = dtW.astype(BF16)
        common[f"dtb_{d}"] = bias_tiles(dtb, 8)
        common[f"outW_{d}"] = outW.astype(BF16)
        common[f"outb_{d}"] = bias_tiles(outb, 4)
    common["fusW"] = np.asarray(inputs["fusion_W"], np.float32).astype(BF16)
    common["fusb"] = bias_tiles(np.asarray(inputs["fusion_b"], np.float32), 4)

    p = np.arange(128)
    e128 = np.zeros((128, 16 * 128), np.float32)
    sel128 = np.zeros((128, 16 * 128), np.float32)
    for t in range(16):
        e128[8 * t + p // 16, 128 * t + p] = 1.0
        sel128[p, 128 * t + 8 * t + p // 16] = 1.0
    e16 = np.zeros((16, 128), np.float32)
    e16[p % 16, p] = 1.0
    common["E128m"] = e128
    common["E16m"] = e16
    common["SEL128m"] = sel128.astype(BF16)

    in_maps = []
    for c in range(NCORES):
        m = dict(common)
        r0 = LC * c
        xpad = np.zeros((HALO, D_MODEL), np.float32)
        lo, hi = max(0, r0 - 3), min(L, r0 + LC + 3)
        xpad[lo - (r0 - 3): hi - (r0 - 3)] = x[lo:hi]
        m["xT"] = np.ascontiguousarray(xpad.T).astype(BF16)
        A_sh = A[128 * c:128 * (c + 1)]                      # (128, 16)
        m["Alan"] = np.ascontiguousarray(
            A_sh.reshape(16, 8, 16).transpose(1, 2, 0).reshape(128, NT))
        m["Dpl"] = np.ascontiguousarray(Dp[128 * c:128 * (c + 1)].reshape(128, 1))
        in_maps.append(m)
    return in_maps


_CACHE = {}


def kernel(**inputs):
    from concourse.bass_utils import run_bass_kernel_spmd
    if "nc" not in _CACHE:
        _CACHE["nc"] = build_bass()
    nc = _CACHE["nc"]
    in_maps = make_in_maps(inputs)
    res = run_bass_kernel_spmd(nc, in_maps, list(range(NCORES)))
    outs = [res.results[c]["outT"] for c in range(NCORES)]
    full = np.concatenate(outs, axis=1)      # (512, 2048)
    return np.ascontiguousarray(full.T).astype(np.float32)
